# revision 1
# baseline (speedup 1.0000x reference)
"""Trainium2 Bass kernel for LoRALinear: out = x @ W^T + bias + scaling * (x @ A^T) @ B^T.

Problem shapes (hardcoded): x [4, 2048, 4096] f32, weight [4096, 4096] f32,
bias [4096] f32, lora_A [16, 4096] f32, lora_B [4096, 16] f32, scaling = 2.0.

Strategy: pure data-parallel over the 8192 token rows across 8 NeuronCores
(1024 rows each, no collectives). Host-side prep folds the LoRA update into
the weight (W_eff = W + scaling * B @ A — exact in fp32, then one fp16
round, which is at least as accurate as rounding W/A/B separately), and
transposes + casts operands to fp16 so the contraction dim (d_in) lands on
SBUF partitions with contiguous DMA runs. The matmul runs on the PE array in
fp16 with fp32 PSUM accumulation.

Per core: out[1024, 4096] = xT.T @ wT + bias, with
  - xT [4096, 1024] fp16 resident in SBUF (8 MiB),
  - wT [4096, 4096] fp16 streamed in 512-wide column slices (double-buffered),
  - bias folded in as a K=1 epilogue matmul (ones row-vector x bias slice),
  - LoRA pre-folded into the weight on host (W_eff = W + 2 B A).
"""

import numpy as np

import concourse.mybir as mybir
import concourse.tile as tile
from concourse import bacc, bass_utils

N_CORES = 8
B, S, D_IN, D_OUT, R = 4, 2048, 4096, 4096, 16
SCALING = 2.0
M_TOTAL = B * S            # 8192
M_CORE = M_TOTAL // N_CORES  # 1024
P = 128
KO = D_IN // P             # 32 contraction tiles
N_SLICE = 512
N_SLICES = D_OUT // N_SLICE  # 8
M_TILES = M_CORE // P        # 8
F16 = mybir.dt.float16
F32 = mybir.dt.float32


def build_nc(reps: int = 1, col_split: bool = False, out_mode: str = 'dve', dt16=None):
    """Build and compile the per-core Bass program. reps>1 wraps the whole
    body in a hardware For_i loop (used only for timing runs)."""
    if dt16 is None:
        dt16 = F16
    nc = bacc.Bacc("TRN2", target_bir_lowering=False, debug=False,
                   num_devices=N_CORES)

    xT_d = nc.dram_tensor("xT", [D_IN, M_CORE], dt16, kind="ExternalInput")
    wT_d = nc.dram_tensor("wT", [D_IN, D_OUT], dt16, kind="ExternalInput")
    bias_d = nc.dram_tensor("bias", [1, D_OUT], dt16, kind="ExternalInput")
    out_d = nc.dram_tensor("out", [M_CORE, D_OUT], F32, kind="ExternalOutput")

    xT_r = xT_d.ap().rearrange("(ko p) m -> p ko m", p=P)    # [128, 32, 1024]
    wT_r = wT_d.ap().rearrange("(ko p) n -> p ko n", p=P)    # [128, 32, 4096]
    out_r = out_d.ap().rearrange("(mt p) n -> mt p n", p=P)  # [8, 128, 4096]

    with tile.TileContext(nc) as tc:
        with (
            tc.tile_pool(name="xp", bufs=1) as x_pool,
            tc.tile_pool(name="wp", bufs=2) as w_pool,
            tc.tile_pool(name="cst", bufs=1) as c_pool,
            tc.tile_pool(name="op", bufs=4) as o_pool,
            tc.tile_pool(name="ps", bufs=4, space="PSUM") as ps_pool,
        ):
            def body(_i=None):
                x_sb = x_pool.tile([P, KO, M_CORE], dt16)
                for i in range(8):
                    nc.sync.dma_start(
                        x_sb[:, i * 4:(i + 1) * 4, :],
                        xT_r[:, i * 4:(i + 1) * 4, :])
                bias_sb = c_pool.tile([1, D_OUT], dt16)
                nc.sync.dma_start(bias_sb[:], bias_d.ap())
                ones_sb = c_pool.tile([1, M_CORE], dt16)
                nc.any.memset(ones_sb[:], 1.0)

                for n in range(N_SLICES):
                    w_sb = w_pool.tile([P, KO, N_SLICE], dt16)
                    w_chunks = 8 if n == 0 else 4
                    for i in range(w_chunks):
                        cw = KO // w_chunks
                        nc.sync.dma_start(
                            w_sb[:, i * cw:(i + 1) * cw, :],
                            wT_r[:, i * cw:(i + 1) * cw,
                                 n * N_SLICE:(n + 1) * N_SLICE])
                    for mt in range(M_TILES):
                        ps = ps_pool.tile([P, N_SLICE], F32)
                        for k in range(KO):
                            if col_split:
                                # two concurrent M=64 col-group matmuls:
                                # the weight load of one group overlaps the
                                # other group's compute (LDWEIGHTS is
                                # otherwise serial with the matmul stream).
                                for j in range(2):
                                    nc.tensor.matmul(
                                        ps[64 * j:64 * (j + 1), :],
                                        x_sb[:, k, mt * P + 64 * j:
                                             mt * P + 64 * (j + 1)],
                                        w_sb[:, k, :],
                                        start=(k == 0), stop=False,
                                        tile_position=(0, 64 * j))
                            else:
                                nc.tensor.matmul(
                                    ps[:],
                                    x_sb[:, k, mt * P:(mt + 1) * P],
                                    w_sb[:, k, :],
                                    start=(k == 0), stop=False)
                        # bias epilogue: K=1 ones-row x bias slice
                        nc.tensor.matmul(
                            ps[:],
                            ones_sb[:, mt * P:(mt + 1) * P],
                            bias_sb[:, n * N_SLICE:(n + 1) * N_SLICE],
                            start=False, stop=True)
                        if out_mode == 'psum_dma':
                            nc.sync.dma_start(
                                out_r[mt, :, n * N_SLICE:(n + 1) * N_SLICE],
                                ps[:])
                        else:
                            o_sb = o_pool.tile([P, N_SLICE], F32)
                            if out_mode == 'dve':
                                nc.vector.tensor_copy(o_sb[:], ps[:])
                            else:
                                nc.any.tensor_copy(o_sb[:], ps[:])
                            nc.sync.dma_start(
                                out_r[mt, :, n * N_SLICE:(n + 1) * N_SLICE],
                                o_sb[:])

            if reps == 1:
                body()
            else:
                with tc.For_i(0, reps, 1) as i:
                    body(i)

    nc.compile()
    return nc


_NC_CACHE = {}


def _get_nc(reps: int = 1, col_split: bool = False, out_mode: str = 'dve', dt16=None):
    key = (reps, col_split, out_mode, str(dt16))
    if key not in _NC_CACHE:
        _NC_CACHE[key] = build_nc(reps, col_split, out_mode, dt16)
    return _NC_CACHE[key]


def prep_in_maps(x, weight, bias, lora_A, lora_B):
    """Host-side shard + pack: returns in_maps for the 8 cores."""
    xf = np.asarray(x, dtype=np.float32).reshape(M_TOTAL, D_IN)
    w_eff = np.asarray(weight, dtype=np.float32) + SCALING * (
        np.asarray(lora_B, dtype=np.float32) @ np.asarray(lora_A, dtype=np.float32))
    wT = np.ascontiguousarray(w_eff.T).astype(np.float16)
    bias1 = np.asarray(bias, dtype=np.float32).astype(np.float16).reshape(1, D_OUT)
    in_maps = []
    for c in range(N_CORES):
        xT_c = np.ascontiguousarray(
            xf[c * M_CORE:(c + 1) * M_CORE].T).astype(np.float16)
        in_maps.append({"xT": xT_c, "wT": wT, "bias": bias1})
    return in_maps


def kernel(x, weight, bias, lora_A, lora_B):
    nc = _get_nc(1)
    in_maps = prep_in_maps(x, weight, bias, lora_A, lora_B)
    res = bass_utils.run_bass_kernel_spmd(nc, in_maps, core_ids=list(range(N_CORES)))
    out = np.concatenate([res.results[c]["out"] for c in range(N_CORES)], axis=0)
    return out.reshape(B, S, D_OUT)

